# revision 59
# baseline (speedup 1.0000x reference)
"""AttnBlock (GroupNorm -> single-head attention over H*W -> proj -> residual)
for Trainium2, 8 NeuronCores via SPMD.

Sharding: core = b*4 + qi (b = batch 0/1, qi = query-quarter 0..3).  The host
permutes each core's x columns so its query quarter is always column-chunk 0
(attention + GN stats are permutation-invariant over the key axis), so one
compiled kernel serves all 8 cores.

v3: every large matmul runs as fp8e4 (e4m3) with MatmulPerfMode.DoubleRow
(K=256 per instruction, 0.5 cycles/row).  Each fp8 tensor carries a
power-of-2 pre-scale placing it in e4m3's normal range (wq^T wk entries are
~2^-9 -- subnormal unscaled); inverse scales fold into activation/
tensor_scalar operands.  wqk = wq^T wk ships as a two-term fp8 sum
(value + quantization residual): it is the dominant quantization error
source.  Scores stay in [J, I] layout; the softmax denominator comes from an
all-ones fp8 DoubleRow matmul broadcasting z to all 128 partitions.  The
j-side GN affine cancels in softmax; the i-side affine folds into the qk
bias.  O accumulates over the full 4096-key sequence in PSUM.

Scheduling: one packed DMA per weight class (tiny SWDGE DMAs would occupy
the Pool engine for ~30us); score j-pairs land in 2-bank PSUM slots rotated
through a 2-buffer pool so one fused exp covers each pair, with every small
PSUM scratch riding the same rotation; qk8/xqs split per i-half so phase-B
readers don't wait on phase-C writers; elementwise work spreads over ACT
(exp, query casts, requants), DVE (stats, V-copies, normalize, residual),
Pool (x->fp8 copies, V-weight/query casts, ic0 residual).  Verified on HW:
rel err 7.2e-3, TimelineSim 96.9us (baseline fp32r kernel: 229.3us).
"""
import sys

sys.path.insert(0, '/opt/trn_rl_repo')

import numpy as np
import ml_dtypes

C = 512
NG = 32
EPS = 1e-6
B = 2
N = 4096          # H*W
NQ = 1024         # query quarter
NCT = 4           # C // 128
NCP = 2           # C // 256 (DoubleRow c-pairs)
NJP = 16          # j-pairs of 256

SHIFT = 2.5
A_WQK = 1024.0
A_XJ = 1.0        # A_XJ * A_WV == A_V so the V cast is a pure copy
A_XQS = 16.0
A_QK = 128.0
A_WV = 16.0
A_V = 16.0
A_O = 16.0        # == A_V keeps the 1/z fold scale-free
A_T = 256.0
A_BVT = 256.0
A_WO = 64.0

_cache = {}


def _legalize_waits(nc, mybir):
    """Codegen allows exactly ONE sync wait per instruction. Hoist excess
    waits onto preceding same-engine NoOps (semantics preserving)."""
    gen = 0
    for f in nc.m.functions:
        for bb in f.blocks:
            insts = list(bb.instructions)
            out = []
            changed = False
            for inst in insts:
                si = inst.sync_info
                waits = list(si.on_wait) if si and si.on_wait else []
                if len(waits) > 1:
                    for w in waits[:-1]:
                        gen += 1
                        nop = mybir.InstNoOp(
                            name=f"waitnop_{gen}", ins=[], outs=[],
                            engine=inst.engine)
                        nop.sync_info = mybir.SyncInfo(on_wait=[w], on_update=[])
                        out.append(nop)
                    inst.sync_info = mybir.SyncInfo(
                        on_wait=[waits[-1]],
                        on_update=list(si.on_update) if si and si.on_update else [])
                    changed = True
                out.append(inst)
            if changed:
                bb.instructions = out


def _build():
    import concourse.bass as bass
    import concourse.tile as tile
    from concourse import mybir
    from contextlib import ExitStack

    f32r = mybir.dt.float32r
    f32 = mybir.dt.float32
    f8 = mybir.dt.float8e4
    bf = mybir.dt.bfloat16
    u8 = mybir.dt.uint8
    u16 = mybir.dt.uint16
    AF = mybir.ActivationFunctionType
    DR = mybir.MatmulPerfMode.DoubleRow
    MUL = mybir.AluOpType.mult
    ADD = mybir.AluOpType.add

    nc = bass.Bass(trn_type="TRN2", target_bir_lowering=False, debug=False)

    x = nc.dram_tensor("x", [C, N], f32, kind="ExternalInput").ap()
    # packed fp8 weights: planes 0-3 wqk8a, 4-7 wqk8b, 8-11 wo8, 12-13 ones
    w8pk = nc.dram_tensor("w8pk", [128, 14, 512], u8, kind="ExternalInput").ap()
    wvpk = nc.dram_tensor("wvpk", [128, 4, 512], u16, kind="ExternalInput").ap()
    # packed per-channel consts: cols 0-3 16*gamma, 4-7 gamma/128,
    # 8-11 16*beta, 12-15 A_QK*hq/16, 16-19 bv, 20-23 bo, 24-31 gmask
    blpk = nc.dram_tensor("blpk", [128, 32], f32, kind="ExternalInput").ap()
    bmask = nc.dram_tensor("bmask", [8, 128], f32, kind="ExternalInput").ap()
    out = nc.dram_tensor("out", [C, NQ], f32, kind="ExternalOutput").ap()

    dma = nc.sync.dma_start
    qi_ch = 0  # host permutes x so the query quarter is chunk 0

    with tile.TileContext(nc) as tc, ExitStack() as top:
        consts = top.enter_context(tc.tile_pool(name="consts", bufs=1))
        xpool = top.enter_context(tc.tile_pool(name="xpool", bufs=1))
        x8pool = top.enter_context(tc.tile_pool(name="x8pool", bufs=1))
        wpool = top.enter_context(tc.tile_pool(name="wpool", bufs=1))
        w8pool = top.enter_context(tc.tile_pool(name="w8pool", bufs=1))
        qk8p = top.enter_context(tc.tile_pool(name="qk8p", bufs=1))
        xqsp = top.enter_context(tc.tile_pool(name="xqsp", bufs=1))
        v8p = top.enter_context(tc.tile_pool(name="v8p", bufs=1))
        ptp = top.enter_context(tc.tile_pool(name="ptp", bufs=1))
        spool = top.enter_context(tc.tile_pool(name="spool", bufs=1))
        rp = top.enter_context(tc.tile_pool(name="rp", bufs=2))
        osbp = top.enter_context(tc.tile_pool(name="osbp", bufs=2))
        outp = top.enter_context(tc.tile_pool(name="outp", bufs=4))
        # PSUM: 4 banks (V/O pairs) + 4 banks (2 x 2-bank score slots);
        # every small scratch rides the score-slot rotation
        ps_v = top.enter_context(tc.tile_pool(name="ps_v", bufs=2, space="PSUM"))
        ps_st = top.enter_context(tc.tile_pool(name="ps_st", bufs=2, space="PSUM"))

        def qtile(shape, name):
            return ps_st.tile(shape, f32, tag="st", name=name)

        def vtile(name):
            return ps_v.tile([128, 2, 512], f32, tag="pvt", name=name)

        def sttile(name):
            return ps_st.tile([128, 1024], f32, tag="st", name=name)

        # ---- packed constant tiles (DMAs emitted after x below) ----
        ballc = consts.tile([128, 32], f32, tag="ballc", name="ballc")
        bm = consts.tile([8, 128], f32, tag="bm", name="bm")
        w8all = w8pool.tile([128, 14, 512], f8, tag="w8all", name="w8all")
        gam16 = [ballc[:, i:i + 1] for i in range(4)]
        gamdq = [ballc[:, 4 + i:5 + i] for i in range(4)]
        bet16 = [ballc[:, 8 + i:9 + i] for i in range(4)]
        hqc16 = [ballc[:, 12 + i:13 + i] for i in range(4)]
        bvc = [ballc[:, 16 + i:17 + i] for i in range(4)]
        boc = [ballc[:, 20 + i:21 + i] for i in range(4)]
        gm = ballc[:, 24:32]
        wqa = lambda p: w8all[:, 2 * p:2 * p + 2, :]          # noqa: E731
        wqb = lambda p: w8all[:, 4 + 2 * p:6 + 2 * p, :]      # noqa: E731
        wo_ = lambda p: w8all[:, 8 + 2 * p:10 + 2 * p, :]     # noqa: E731
        ones8 = w8all[:, 12:14, 0:128]
        epst = consts.tile([128, 1], f32, tag="epst", name="epst")
        nc.vector.memset(epst[:], EPS)
        shiftb = consts.tile([128, 1], f32, tag="shiftb", name="shiftb")
        nc.vector.memset(shiftb[:], -SHIFT)

        # ---- x resident (DMA) + fp8 copy (Pool) + stats (DVE) ----
        x_t = [[xpool.tile([128, 1024], f32, tag=f"x{ci}_{ch}", name=f"x{ci}_{ch}")
                for ch in range(4)] for ci in range(NCT)]
        x8t = [[x8pool.tile([128, 2, 1024], f8, tag=f"x8{p}_{ch}", name=f"x8{p}_{ch}")
                for ch in range(4)] for p in range(NCP)]
        stats3 = [spool.tile([128, 8, 6], f32, tag=f"st3s{i}", name=f"st3s{i}")
                  for i in range(NCT)]
        stats2 = [spool.tile([128, 2], f32, tag=f"st2{i}", name=f"st2{i}") for i in range(NCT)]
        ssum = qtile([8, 8], "ssum")
        dma(ballc[:], blpk)
        dma(bm[:], bmask)
        # the final chunk splits into two half-tiles so its first bn_stats
        # starts one DMA-half earlier (it gates the whole GN scale chain)
        xl = [xpool.tile([128, 512], f32, tag=f"xl{h}", name=f"xl{h}")
              for h in range(2)]
        for ci in range(NCT):
            for ch in range(4):
                dmax = dma
                last = (ci == NCT - 1 and ch == 3)
                if last:
                    for h in range(2):
                        dmax(xl[h][:], x[ci * 128:(ci + 1) * 128,
                                         ch * 1024 + h * 512:ch * 1024 + (h + 1) * 512])
                        nc.vector.bn_stats(out=stats3[ci][:, ch * 2 + h, :],
                                           in_=xl[h][:])
                        nc.gpsimd.tensor_copy(
                            x8t[ci // 2][ch][:, ci % 2, h * 512:(h + 1) * 512],
                            xl[h][:])
                    continue
                dmax(x_t[ci][ch][:], x[ci * 128:(ci + 1) * 128,
                                       ch * 1024:(ch + 1) * 1024])
                nc.gpsimd.tensor_copy(x8t[ci // 2][ch][:, ci % 2, :],
                                      x_t[ci][ch][:])
                for h in range(2):
                    nc.vector.bn_stats(
                        out=stats3[ci][:, ch * 2 + h, :],
                        in_=x_t[ci][ch][:, h * 512:(h + 1) * 512])
            # per-ci stats head pipelined against remaining x DMAs
            mv = spool.tile([128, 2], f32, tag="mv", name="mv")
            nc.vector.bn_aggr(out=mv[:], in_=stats3[ci][:, :, :])
            m2 = spool.tile([128, 1], f32, tag="m2", name="m2")
            nc.vector.tensor_mul(m2[:], mv[:, 0:1], mv[:, 0:1])
            nc.vector.tensor_add(stats2[ci][:, 1:2], mv[:, 1:2], m2[:])
            nc.vector.tensor_copy(stats2[ci][:, 0:1], mv[:, 0:1])
            nc.tensor.matmul(ssum[0:8, 2 * ci:2 * ci + 2], gm,
                             stats2[ci][:], start=True, stop=True)
        wvall = wpool.tile([128, 4, 512], bf, tag="wvall", name="wvall")
        dma(wvall[:], wvpk.bitcast(bf))
        wv_t = [wvall[:, i, :] for i in range(NCT)]
        dma(w8all[:], w8pk.bitcast(f8))

        # ---- P1: group stats -> per-channel scale/shift ----
        sg = spool.tile([8, 8], f32, tag="sg", name="sg")
        nc.scalar.activation(sg[:], ssum[:], AF.Copy)
        m2g = spool.tile([8, 4], f32, tag="m2g", name="m2g")
        nc.vector.tensor_mul(m2g[:], sg[:, 0:8:2], sg[:, 0:8:2])
        varg = spool.tile([8, 4], f32, tag="varg", name="varg")
        nc.vector.tensor_sub(varg[:], sg[:, 1:8:2], m2g[:])
        sq = spool.tile([8, 4], f32, tag="sq", name="sq")
        nc.scalar.activation(sq[:], varg[:], AF.Sqrt, bias=epst[0:8, :],
                             scale=1.0)
        r0 = spool.tile([8, 4], f32, tag="r0", name="r0")
        nc.vector.reciprocal(r0[:], sq[:])
        mrstd = spool.tile([8, 8], f32, tag="mrstd", name="mrstd")
        nc.vector.tensor_copy(mrstd[:, 0:8:2], sg[:, 0:8:2])
        nc.vector.tensor_copy(mrstd[:, 1:8:2], r0[:])
        s16 = [spool.tile([128, 1], f32, tag=f"s16_{i}", name=f"s16_{i}") for i in range(NCT)]
        sdq = [spool.tile([128, 1], f32, tag=f"sdq{i}", name=f"sdq{i}") for i in range(NCT)]
        t2b = [spool.tile([128, 2], bf, tag=f"t2b{i}", name=f"t2b{i}") for i in range(NCT)]
        t16 = [spool.tile([128, 1], f32, tag=f"t16_{i}", name=f"t16_{i}") for i in range(NCT)]
        suq = [spool.tile([128, 1], f32, tag=f"suq{i}", name=f"suq{i}") for i in range(NCT)]
        for ci in range(NCT):
            pc_ps = qtile([128, 2], f"pc{ci}")
            nc.tensor.matmul(pc_ps[:], bm[:], mrstd[:, 2 * ci:2 * ci + 2],
                             start=True, stop=True)
            perch = spool.tile([128, 2], f32, tag=f"pch{ci}", name=f"pch{ci}")
            nc.scalar.activation(perch[:], pc_ps[:], AF.Copy)
            # s16 = A_XQS*s (== A_WV*s), sdq = s*A_QK/(A_WQK*A_XQS): host
            # prescales gamma so each is one ACT op off the rstd column
            nc.scalar.activation(s16[ci][:], perch[:, 1:2], AF.Identity,
                                 scale=gam16[ci])
            nc.scalar.activation(sdq[ci][:], perch[:, 1:2], AF.Identity,
                                 scale=gamdq[ci])
            tmp = spool.tile([128, 1], f32, tag="tmp1", name="tmp1")
            nc.vector.tensor_mul(tmp[:], perch[:, 0:1], s16[ci][:])
            nc.vector.tensor_sub(t16[ci][:], bet16[ci], tmp[:])
            nc.vector.tensor_mul(suq[ci][:], hqc16[ci], s16[ci][:])
            # t2b (bf16 GN shift for the off-critical bvt fold) on Pool
            nc.gpsimd.tensor_scalar_mul(t2b[ci][:, 0:1], t16[ci][:], 1.0 / A_XQS)
            nc.gpsimd.tensor_scalar_mul(t2b[ci][:, 1:2], t16[ci][:], 1.0 / A_XQS)

        # ---- query-side casts + qk projection (per i-half) ----
        xqs = [[xqsp.tile([128, 2, 512], f8, tag=f"xqs{p}_{ih}",
                          name=f"xqs{p}_{ih}") for ih in range(2)]
               for p in range(NCP)]
        qk8 = [[qk8p.tile([128, 2, 512], f8, tag=f"qk8{p}_{ih}",
                          name=f"qk8{p}_{ih}") for ih in range(2)]
               for p in range(NCP)]
        # xqs = A_XQS * (s*xq + t): ih0 cp0 on ACT, cp1 on Pool (parallel
        # head); critical path runs to the first exp
        for ci in range(2):
            nc.scalar.activation(xqs[0][0][:, ci, :],
                                 x_t[ci][qi_ch][:, 0:512],
                                 AF.Identity, bias=t16[ci][:],
                                 scale=s16[ci][:])
        for ci in range(2, NCT):
            nc.gpsimd.tensor_scalar(out=xqs[1][0][:, ci % 2, :],
                                    in0=x_t[ci][qi_ch][:, 0:512],
                                    scalar1=s16[ci][:], scalar2=t16[ci][:],
                                    op0=MUL, op1=ADD)
        # i1-half query cast and V-weight cast on the otherwise-idle Pool
        # engine; DVE carries only qk8-ih1 + the 16 V copies through phase B.
        wv8 = [w8pool.tile([128, 2, 512], f8, tag=f"wv8{p}", name=f"wv8{p}")
               for p in range(NCP)]
        for ci in range(NCT):
            nc.gpsimd.tensor_scalar_mul(wv8[ci // 2][:, ci % 2, :],
                                        wv_t[ci], s16[ci][:])
        for ci in range(NCT):
            nc.gpsimd.tensor_scalar(out=xqs[ci // 2][1][:, ci % 2, :],
                                    in0=x_t[ci][qi_ch][:, 512:1024],
                                    scalar1=s16[ci][:], scalar2=t16[ci][:],
                                    op0=MUL, op1=ADD)

        def qk_mms(ih):
            # pqk rides the 3-bank score-tile rotation (idle pre-scores) so
            # the four mt projections requantize concurrently
            for mt in range(NCT):
                m_sl = slice(mt * 128, (mt + 1) * 128)
                pqk = qtile([128, 512], f"pqk{ih}_{mt}")
                k = 0
                for p in range(NCP):
                    for wt in (wqa, wqb):
                        nc.tensor.matmul(pqk[:], wt(p)[:, :, m_sl],
                                         xqs[p][ih][:],
                                         start=(k == 0), stop=(k == 3),
                                         perf_mode=DR)
                        k += 1
                if ih == 1 or mt < 2:
                    nc.scalar.activation(qk8[mt // 2][ih][:, mt % 2, :],
                                         pqk[:], AF.Identity,
                                         bias=suq[mt][:], scale=sdq[mt][:])
                else:
                    nc.vector.tensor_scalar(
                        out=qk8[mt // 2][ih][:, mt % 2, :], in0=pqk[:],
                        scalar1=sdq[mt][:], scalar2=suq[mt][:],
                        op0=MUL, op1=ADD)

        # ---- V phase + scores/exp, interleaved emission ----
        v8 = [v8p.tile([128, 2, 512], f8, tag=f"v8_{jp}", name=f"v8_{jp}")
              for jp in range(NJP)]
        pt8 = [[ptp.tile([128, 2, 512], f8, tag=f"pt{ic}_{jp}",
                         name=f"pt{ic}_{jp}") for jp in range(NJP)]
               for ic in range(2)]

        pvts = {}

        def v_mms(jp):
            j0 = jp * 256
            ch, jj0 = j0 // 1024, j0 % 1024
            pvt = pvts[jp] = vtile(f"pvt{jp}")
            for jt in range(2):
                jsl = slice(jj0 + jt * 128, jj0 + jt * 128 + 128)
                for p in range(NCP):
                    nc.tensor.matmul(pvt[:, jt, :], x8t[p][ch][:, :, jsl],
                                     wv8[p][:], start=(p == 0),
                                     stop=(p == NCP - 1), perf_mode=DR)

        def v_copy(jp):
            nc.vector.tensor_copy(v8[jp][:], pvts.pop(jp)[:])

        def se_jp(ic, jp):
            j0 = jp * 256
            ch, jj0 = j0 // 1024, j0 % 1024
            st = sttile(f"s{ic}_{jp}")
            for jt in range(2):
                jsl = slice(jj0 + jt * 128, jj0 + jt * 128 + 128)
                for p in range(NCP):
                    nc.tensor.matmul(st[:, jt * 512:(jt + 1) * 512],
                                     x8t[p][ch][:, :, jsl],
                                     qk8[p][ic][:],
                                     start=(p == 0), stop=(p == NCP - 1),
                                     perf_mode=DR)
            nc.scalar.activation(pt8[ic][jp][:], st[:], AF.Exp,
                                 bias=shiftb[:], scale=1.0 / (A_XJ * A_QK))

        # ---- bvt / bo folds (PE here; elementwise on DVE) ----
        bvt_s = [spool.tile([128, 2], f32, tag=f"bvs{i}", name=f"bvs{i}") for i in range(NCT)]
        bvt8 = [spool.tile([128, 2, 1], f8, tag=f"bv8{p}", name=f"bv8{p}") for p in range(NCP)]
        bo_s = [spool.tile([128, 1], f32, tag=f"bos{i}", name=f"bos{i}") for i in range(NCT)]
        for mt in range(NCT):
            m_sl = slice(mt * 128, (mt + 1) * 128)
            pv = qtile([128, 2], f"pv{mt}")
            for ci in range(NCT):
                nc.tensor.matmul(pv[:], wv_t[ci][:, m_sl], t2b[ci][:],
                                 start=(ci == 0), stop=(ci == NCT - 1))
            nc.vector.tensor_scalar_add(bvt_s[mt][:], pv[:], bvc[mt])
            nc.vector.tensor_scalar_mul(bvt8[mt // 2][:, mt % 2, :],
                                        bvt_s[mt][:, 0:1], A_BVT)
        for ot in range(NCT):
            o_sl = slice(ot * 128, (ot + 1) * 128)
            pb = qtile([128, 1], f"pb{ot}")
            for p in range(NCP):
                nc.tensor.matmul(pb[:], wo_(p)[:, :, o_sl], bvt8[p][:],
                                 start=(p == 0), stop=(p == NCP - 1),
                                 perf_mode=DR)
            nc.vector.tensor_scalar(out=bo_s[ot][:], in0=pb[:, 0:1],
                                    scalar1=1.0 / (A_WO * A_BVT),
                                    scalar2=boc[ot], op0=MUL, op1=ADD)

        # V lags the score stream by 4 j-pairs so the pvt double-buffer's
        # DVE copies never gate the PE->ACT exp cadence.
        qk_mms(0)
        for jp in range(NJP):
            se_jp(0, jp)
            k = jp - 5
            if k >= 0:
                v_mms(k)
                v_copy(k)
            if jp == 6:
                qk_mms(1)
        for k in range(NJP - 5, NJP):
            v_mms(k)
            v_copy(k)

        # ---- phase C: z(ic0), O(ic0) + scores/exp(ic1) ----
        oAB = [vtile("oA"), vtile("oB")]
        o_ps0 = [oAB[mt // 2][:, mt % 2, :] for mt in range(NCT)]
        z0 = qtile([128, 512], "z0")
        rinv0 = rp.tile([128, 512], f32, tag="rinv", name="rinv0")
        for jp in range(NJP):
            se_jp(1, jp)
            for mt in range(NCT):
                nc.tensor.matmul(o_ps0[mt], v8[jp][:, :, mt * 128:(mt + 1) * 128],
                                 pt8[0][jp][:], start=(jp == 0),
                                 stop=(jp == NJP - 1), perf_mode=DR)
            if jp == 4:
                # emitted mid-C so the PE burst rides C's slack instead of
                # delaying the first ic1 scores at the phase boundary
                for zj in range(NJP):
                    nc.tensor.matmul(z0[:], ones8, pt8[0][zj][:],
                                     start=(zj == 0), stop=(zj == NJP - 1),
                                     perf_mode=DR)
                with nc.allow_low_precision(reason="softmax reciprocal"):
                    nc.vector.reciprocal(rinv0[:], z0[:])

        # ---- phase D: z(ic1), O(ic1), epilogues ----
        zt1 = qtile([128, 512], "zt1")
        for jp in range(NJP):
            nc.tensor.matmul(zt1[:], ones8, pt8[1][jp][:],
                             start=(jp == 0), stop=(jp == NJP - 1),
                             perf_mode=DR)
        rinv1 = rp.tile([128, 512], f32, tag="rinv", name="rinv1")
        with nc.allow_low_precision(reason="softmax reciprocal"):
            nc.vector.reciprocal(rinv1[:], zt1[:])
        oCD = [vtile("oC"), vtile("oD")]
        o_ps1 = [oCD[mt // 2][:, mt % 2, :] for mt in range(NCT)]

        def o1_jp(jp):
            for mt in range(NCT):
                nc.tensor.matmul(o_ps1[mt], v8[jp][:, :, mt * 128:(mt + 1) * 128],
                                 pt8[1][jp][:], start=(jp == 0),
                                 stop=(jp == NJP - 1), perf_mode=DR)

        def epilogue(ic, o_ps, rinv, resid_eng):
            i0 = ic * 512
            osb = [osbp.tile([128, 2, 512], f8, tag=f"osb{p}", name=f"osb{ic}{p}")
                   for p in range(NCP)]
            for mt in range(NCT):
                nc.vector.tensor_mul(osb[mt // 2][:, mt % 2, :],
                                     o_ps[mt], rinv[:])
            for ot in range(NCT):
                o_sl = slice(ot * 128, (ot + 1) * 128)
                f_ps = qtile([128, 512], f"f{ic}_{ot}")
                for p in range(NCP):
                    nc.tensor.matmul(f_ps[:], wo_(p)[:, :, o_sl], osb[p][:],
                                     start=(p == 0), stop=(p == NCP - 1),
                                     perf_mode=DR)
                ot_sb = outp.tile([128, 512], f32, tag="outsb", name="outsb")
                nc.scalar.activation(ot_sb[:], f_ps[:], AF.Identity,
                                     bias=bo_s[ot][:], scale=1.0 / (A_WO * A_O))
                resid_eng.tensor_add(ot_sb[:], ot_sb[:],
                                     x_t[ot][qi_ch][:, i0:i0 + 512])
                dma(out[ot * 128:(ot + 1) * 128, i0:i0 + 512], ot_sb[:])

        for jp in range(5):
            o1_jp(jp)
        epilogue(0, o_ps0, rinv0, nc.gpsimd)
        for jp in range(5, NJP):
            o1_jp(jp)
        epilogue(1, o_ps1, rinv1, nc.vector)

    _legalize_waits(nc, mybir)
    return nc


def kernel(**inputs):
    import concourse.bass  # noqa: F401
    from concourse.bass_utils import run_bass_kernel_spmd

    E4 = ml_dtypes.float8_e4m3
    BF = ml_dtypes.bfloat16

    x = np.asarray(inputs["x"], dtype=np.float32)
    gamma = np.asarray(inputs["gamma"], np.float32)
    beta = np.asarray(inputs["beta"], np.float32)
    wq = np.asarray(inputs["wq"], np.float32)
    bq = np.asarray(inputs["bq"], np.float32)
    wk = np.asarray(inputs["wk"], np.float32)
    wv = np.asarray(inputs["wv"], np.float32)
    wo = np.asarray(inputs["wo"], np.float32)
    bv = np.asarray(inputs["bv"], np.float32)
    bo = np.asarray(inputs["bo"], np.float32)

    Bb, Cc, H, W = x.shape
    scale = Cc ** (-0.5)
    xf = x.reshape(Bb, Cc, H * W)

    def pack_dr(w, alpha):
        """[C, 512] f32 -> [128, 2cp, 2t, 512] e4m3 planes (c = cp*256+t*128+p)."""
        q = np.clip(alpha * w, -240, 240).astype(E4)
        return q.reshape(2, 2, 128, 512).transpose(2, 0, 1, 3)  # [p, cp, t, m]

    wqk_h = scale * (wq.T @ wk)
    a8f = np.clip(A_WQK * wqk_h, -240, 240).astype(E4).astype(np.float32)
    w8pk = np.zeros((128, 14, 512), E4)
    w8pk[:, 0:4, :] = pack_dr(wqk_h, A_WQK).reshape(128, 4, 512)
    w8pk[:, 4:8, :] = (np.clip(A_WQK * wqk_h - a8f, -240, 240).astype(E4)
                       .reshape(2, 2, 128, 512).transpose(2, 0, 1, 3)
                       .reshape(128, 4, 512))
    w8pk[:, 8:12, :] = pack_dr(wo.T, A_WO).reshape(128, 4, 512)
    w8pk[:, 12:14, 0:128] = np.ones((128, 2, 128), np.float32).astype(E4)
    w8pk = np.ascontiguousarray(w8pk).view(np.uint8)

    wvpk = np.ascontiguousarray(
        wv.T.astype(BF).reshape(4, 128, 512).transpose(1, 0, 2)).view(np.uint16)

    blpk = np.zeros((128, 32), np.float32)
    blpk[:, 0:4] = (A_XQS * gamma).reshape(4, 128).T
    blpk[:, 4:8] = (gamma * (A_QK / (A_WQK * A_XQS))).reshape(4, 128).T
    blpk[:, 8:12] = (A_XQS * beta).reshape(4, 128).T
    blpk[:, 12:16] = ((A_QK / A_XQS) * scale * (wk.T @ bq)).reshape(4, 128).T
    blpk[:, 16:20] = bv.reshape(4, 128).T
    blpk[:, 20:24] = bo.reshape(4, 128).T
    for p in range(128):
        blpk[p, 24 + p // 16] = 1.0 / 16.0
    bmask = np.zeros((8, 128), np.float32)
    for p in range(128):
        bmask[p // 16, p] = 1.0

    common = {"w8pk": w8pk, "wvpk": wvpk, "blpk": blpk, "bmask": bmask}

    if "nc" not in _cache:
        _cache["nc"] = _build()
    nc = _cache["nc"]

    in_maps = []
    for core in range(8):
        b, qi = core // 4, core % 4
        xb = xf[b]
        q0 = qi * NQ
        xperm = np.ascontiguousarray(np.concatenate(
            [xb[:, q0:q0 + NQ], xb[:, :q0], xb[:, q0 + NQ:]], axis=1))
        in_maps.append({"x": xperm, **common})

    res = run_bass_kernel_spmd(nc, in_maps, core_ids=list(range(8)))
    outf = np.empty((Bb, Cc, H * W), np.float32)
    for core in range(8):
        b, qi = core // 4, core % 4
        outf[b][:, qi * NQ:(qi + 1) * NQ] = res.results[core]["out"]
    return outf.reshape(Bb, Cc, H, W)
